# revision 7
# baseline (speedup 1.0000x reference)
"""Local (windowed) attention Trainium2 Bass kernel.

Problem: q,k,v [8, 8, 4096, 64] fp32; window 128, look_backward 1, pad -1.0.
out[b,h,w,i,:] = softmax(scale * q_wi . [k_{w-1}; k_w]) @ [v_{w-1}; v_w]
(with window -1 = all -1.0 pad values, which DO enter the softmax).

Sharding: data-parallel over flat batch*heads (64) -> 8 heads per core.

Per-core layouts (prepared host-side):
  qT : [4, 128, 4096]  float16 - head pair stacked on partitions (d=64 each),
                                 free axis = 4096 queries (d-major transposed)
  kT : [4, 128, 4224]  float16 - same, with one pad chunk (128 keys of -1.0)
                                 prepended -> 33 chunks of 128 keys
  v  : [8, 128, 33, 65] float16 - per head; partition = key-within-chunk,
                                 pad chunk prepended; col 64 = 1.0 (ones
                                 column yields softmax denominator l)
  out: [4, 128, 32, 2, 64] float16 - partition = query-within-window,
                                 head interleaved (de-interleaved on host)

Device pipeline per head pair, per key-chunk group (2 chunks):
  MM1 (fp16): scoresT[j, i] for the <=2 windows attending each chunk;
              heads of a pair alternate PE row groups (base partition 0/64);
              each PSUM bank only ever sees one weight base partition
              (mixing row-group bases within a bank hard-crashes the device).
              Scores pool bufs=3 so MM1 runs up to 2 groups ahead of exp.
  ACT exp (scale=1/8): one 1024-col activation per group, psum -> fp16 P.
              The ACT engine is the roofline for this problem (~74us busy:
              8.4M exps/core at 1 elem/lane/cycle @ 1.2 GHz) - everything
              else is arranged to keep it saturated and off other engines.
  MM2 (fp16): out_w[i, 0:65] += P_blockT @ v_aug[p] (col 64 accumulates l),
              accumulated in psum tiles [128, 2 windows, 2 heads, 65] - one
              bank per 2x2 block (pool bufs=2 -> only 2 banks).
  DVE: per 2x2 block, reciprocal(l) straight from psum then one broadcast
       tensor-multiply psum -> fp16 staging (no intermediate copy); one
       contiguous DMA store per 8 windows.
DMA: k/q slice loads issued from SP (nc.sync), v loads and output stores
     from Pool (nc.gpsimd) - a single issuing engine (~600ns per HWDGE
     start) otherwise rate-limits aggregate DMA to ~250 GB/s.

Accuracy: ~6e-4 relative (fp16 operand rounding; exact fp32 PSUM
accumulation; the 1/8 softmax scale keeps logit perturbation ~4e-4).
"""

import os
import sys

for _p in ("/opt/trn_rl_repo", "/opt/pypackages"):
    if os.path.isdir(_p) and _p not in sys.path:
        sys.path.append(_p)

import numpy as np

import concourse.mybir as mybir
import concourse.tile as tile
from concourse import bacc
from concourse.bass_utils import run_bass_kernel_spmd

B, H, N, D = 8, 8, 4096, 64
WS = 128                 # window size
W = N // WS              # 32 windows
C = W + 1                # 33 key chunks incl. pad chunk
NC = 8                   # cores
HPC = (B * H) // NC      # 8 heads per core
PAIRS = HPC // 2         # 4 head pairs per core
SCALE = float(D) ** -0.5

MM_DT = mybir.dt.float16
GROUP = 2                # key chunks per exp batch
WB = 2                   # windows per out-psum block
SB = 8                   # windows per staged store

_NC_CACHE = {}


def build_nc(pairs=PAIRS, w=W):
    c = w + 1
    n = w * WS
    nc = bacc.Bacc("TRN2", target_bir_lowering=False)
    qT = nc.dram_tensor("qT", [pairs, 128, n], MM_DT, kind="ExternalInput")
    kT = nc.dram_tensor("kT", [pairs, 128, c * WS], MM_DT, kind="ExternalInput")
    vv = nc.dram_tensor("v", [2 * pairs, 128, c, D + 1], MM_DT, kind="ExternalInput")
    out = nc.dram_tensor("out", [pairs, 128, w, 2, D], MM_DT,
                         kind="ExternalOutput")

    f32 = mybir.dt.float32
    Exp = mybir.ActivationFunctionType.Exp

    with tile.TileContext(nc) as tc:
        with (
            tc.tile_pool(name="qk", bufs=3) as qk_pool,
            tc.tile_pool(name="vp", bufs=6) as v_pool,
            tc.tile_pool(name="pt", bufs=3) as pt_pool,
            tc.tile_pool(name="st", bufs=2) as st_pool,
            tc.tile_pool(name="rc", bufs=2) as rc_pool,
            tc.tile_pool(name="ps_s", bufs=3, space="PSUM") as ps_s,
            tc.tile_pool(name="ps_o", bufs=2, space="PSUM") as ps_o,
        ):
            pending_mm2 = None

            def emit_block_epilogue(ctx, wp):
                # windows {2wp, 2wp+1} of both heads are complete in
                # out_ps[wp]: normalize straight out of psum into the
                # fp16 staging slab; store the slab when it fills.
                pair, out_ps, stg = ctx["pair"], ctx["out_ps"], ctx["stg"]
                t = out_ps.pop(wp)
                sl = (WB * wp) // SB
                if sl not in stg:
                    stg[sl] = st_pool.tile([128, SB, 2, D], MM_DT,
                                           tag="stg",
                                           name=f"st_{pair}_{sl}")
                recip = rc_pool.tile([128, WB, 2], f32, tag="recip",
                                     name=f"rc_{pair}_{wp}")
                nc.vector.reciprocal(recip, t[:, :, :, D])
                bw = (WB * wp) % SB
                nc.vector.tensor_mul(
                    stg[sl][:, bw:bw + WB],
                    t[:, :, :, 0:D],
                    recip[:, :, :, None].to_broadcast((128, WB, 2, D)),
                )
                if bw + WB == SB:
                    w0 = sl * SB
                    nc.gpsimd.dma_start(out[pair][:, w0:w0 + SB],
                                        stg.pop(sl))

            def do_mm2s(ctx, chunks, pt):
                # PSUM has_written bits: start=True clears the WHOLE
                # bank's bits, so with 4 accumulation slices (2 windows
                # x 2 heads) sharing a bank, only the tile's very first
                # matmul may use start=True.  Later first-writes use
                # flags=0, which overwrites where the bit is unset and
                # accumulates where it is set - per-element "start".
                pair, out_ps, vts = ctx["pair"], ctx["out_ps"], ctx["vts"]
                for s, p in enumerate(chunks):
                    done_wp = None
                    for h in range(2):
                        col = h * (GROUP * 256) + s * 256
                        if p >= 1:
                            # window p-1 self-contribution (last)
                            wi = p - 1
                            t = out_ps[wi // WB]
                            nc.tensor.matmul(
                                t[:, wi % WB, h, :],
                                pt[:, col:col + WS],
                                vts[h][:, p, :],
                                start=False, stop=True,
                            )
                            if wi % WB == WB - 1 and h == 1:
                                done_wp = wi // WB
                        if p <= w - 1:
                            # window p prev-contribution (first)
                            bcol = col + (WS if p >= 1 else 0)
                            fresh = p % WB == 0 and h == 0
                            if fresh:
                                out_ps[p // WB] = ps_o.tile(
                                    [128, WB, 2, D + 1], f32, tag="out",
                                    name=f"ops_{pair}_{p // WB}")
                            t = out_ps[p // WB]
                            nc.tensor.matmul(
                                t[:, p % WB, h, :],
                                pt[:, bcol:bcol + WS],
                                vts[h][:, p, :],
                                start=fresh, stop=False,
                            )
                    if done_wp is not None:
                        emit_block_epilogue(ctx, done_wp)

            for pair in range(pairs):
                qt = qk_pool.tile([128, n], MM_DT, tag="qT")
                kt = qk_pool.tile([128, c * WS], MM_DT, tag="kT")

                # k/q slice loads on SP; v on Pool so HWDGE issue (~600ns
                # per start) is split across two queues.  First slices are
                # small so MM1 of group 0 starts ASAP.
                if pair == 0:
                    ksl = [(0, 512), (512, 1024), (1024, 2112), (2112, c * WS)]
                    qsl = [(0, 512), (512, 1024), (1024, 2048), (2048, n)]
                else:
                    ksl = [(0, 2112), (2112, c * WS)]
                    qsl = [(0, 2048), (2048, n)]

                def load_kq(i):
                    if i < len(ksl):
                        a, b = ksl[i]
                        nc.sync.dma_start(kt[:, a:b], kT[pair][:, a:b])
                        a, b = qsl[i]
                        nc.sync.dma_start(qt[:, a:b], qT[pair][:, a:b])

                load_kq(0)
                load_kq(1)
                vts = [v_pool.tile([128, c, D + 1], MM_DT, tag="v",
                                   name=f"v_{pair}_{h}") for h in range(2)]
                ch = c // 2
                for h in range(2):
                    nc.gpsimd.dma_start(vts[h][:, 0:ch], vv[2 * pair + h][:, 0:ch])
                load_kq(2)
                for h in range(2):
                    nc.gpsimd.dma_start(vts[h][:, ch:], vv[2 * pair + h][:, ch:])
                load_kq(3)

                ctx = {"pair": pair, "vts": vts, "out_ps": {}, "stg": {}}

                groups = [list(range(g, min(g + GROUP, c)))
                          for g in range(0, c, GROUP)]
                for chunks in groups:
                    ps = ps_s.tile([128, GROUP * 2 * 256], f32, tag="scores")
                    runs = []  # written (col, n) regions
                    for s, p in enumerate(chunks):
                        qlo = max(0, (p - 1) * WS)
                        qhi = min(n, (p + 1) * WS)
                        if p == 0:
                            qhi = min(n, 2 * WS)  # avoid garbage: fill 256
                        nq = qhi - qlo
                        for h in range(2):
                            col = h * (GROUP * 256) + s * 256
                            nc.tensor.matmul(
                                ps[:, col:col + nq],
                                kt[64 * h:64 * h + 64, p * WS:(p + 1) * WS],
                                qt[64 * h:64 * h + 64, qlo:qhi],
                                start=True, stop=True,
                            )
                            runs.append((col, nq))
                    # batched exp: merge adjacent written runs
                    pt = pt_pool.tile([128, GROUP * 2 * 256], MM_DT, tag="pt")
                    merged = []
                    for rcol, rn in sorted(runs):
                        if merged and merged[-1][0] + merged[-1][1] == rcol:
                            merged[-1][1] += rn
                        else:
                            merged.append([rcol, rn])
                    for rcol, rn in merged:
                        nc.scalar.activation(pt[:, rcol:rcol + rn],
                                             ps[:, rcol:rcol + rn],
                                             Exp, scale=SCALE)
                    # MM2s deferred one group for pipelining; the
                    # deferral crosses pair boundaries so the next pair's
                    # MM1s overlap this pair's trailing MM2s on PE.
                    if pending_mm2 is not None:
                        do_mm2s(*pending_mm2)
                    pending_mm2 = (ctx, chunks, pt)
            if pending_mm2 is not None:
                do_mm2s(*pending_mm2)
                pending_mm2 = None

    nc.compile()
    return nc


def _get_nc():
    if "nc" not in _NC_CACHE:
        _NC_CACHE["nc"] = build_nc()
    return _NC_CACHE["nc"]


def _prep_core(qf, kf, vf, lo):
    """Build one core's input dict from flat [64, 4096, 64] fp32 arrays."""
    q8 = qf[lo:lo + HPC]                      # [8, 4096, 64]
    k8 = kf[lo:lo + HPC]
    v8 = vf[lo:lo + HPC]

    qT = np.ascontiguousarray(q8.transpose(0, 2, 1)).reshape(PAIRS, 128, N)
    qT = qT.astype(np.float16)

    pad = np.full((HPC, WS, D), -1.0, dtype=np.float32)
    kp = np.concatenate([pad, k8], axis=1)    # [8, 4224, 64]
    kT = np.ascontiguousarray(kp.transpose(0, 2, 1)).reshape(PAIRS, 128, C * WS)
    kT = kT.astype(np.float16)

    vp = np.concatenate([pad, v8], axis=1)    # [8, 4224, 64]
    ones = np.ones((HPC, C * WS, 1), dtype=np.float32)
    va = np.concatenate([vp, ones], axis=2)   # [8, 4224, 65]
    va = va.reshape(HPC, C, WS, D + 1).transpose(0, 2, 1, 3)  # [8, 128, 33, 65]
    va = np.ascontiguousarray(va).astype(np.float16)

    return {"qT": qT, "kT": kT, "v": va}


def kernel(q, k, v):
    q = np.asarray(q, dtype=np.float32)
    k = np.asarray(k, dtype=np.float32)
    v = np.asarray(v, dtype=np.float32)
    qf = q.reshape(B * H, N, D)
    kf = k.reshape(B * H, N, D)
    vf = v.reshape(B * H, N, D)

    nc = _get_nc()
    in_maps = [_prep_core(qf, kf, vf, HPC * c) for c in range(NC)]
    res = run_bass_kernel_spmd(nc, in_maps, core_ids=list(range(NC)))

    outs = []
    for c in range(NC):
        o = res.results[c]["out"]             # [4, 128, 32, 2, 64] fp16
        # [pair, i, w, h, d] -> [pair, h, w, i, d] -> [8, 4096, 64]
        o = o.transpose(0, 3, 2, 1, 4).reshape(HPC, N, D)
        outs.append(o)
    return np.concatenate(outs, axis=0).reshape(B, H, N, D).astype(np.float32)


if __name__ == "__main__":
    rng = np.random.default_rng(0)
    q = rng.standard_normal((B, H, N, D), dtype=np.float32)
    k = rng.standard_normal((B, H, N, D), dtype=np.float32)
    v = rng.standard_normal((B, H, N, D), dtype=np.float32)
    o = kernel(q, k, v)
    print("out", o.shape, o.dtype, float(np.abs(o).max()))
